# revision 52
# baseline (speedup 1.0000x reference)
"""Trainium2 Bass kernel for nn_Discriminator: MLP + sort-based minibatch discrimination. Self-contained."""
import numpy as np

N = 2048
NROWS = 4
NCOLS = 512


def stages(n=None):
    if n is None:
        n = N
    out = []
    p = 1
    while p < n:
        k = p
        while k >= 1:
            lefts = []
            j = k % p
            while j <= n - 1 - k:
                for i in range(0, min(k, n - j - k)):
                    x = i + j
                    if (x // (2 * p)) == ((x + k) // (2 * p)):
                        lefts.append(x)
                j += 2 * k
            out.append((p, k, np.array(sorted(lefts), dtype=np.int64)))
            k //= 2
        p *= 2
    return out


def runs_of(xs):
    """Compress sorted ints into <=3-level pattern (start, L, s1, c1, s2, c2).
    Returns single tuple or None."""
    xs = np.asarray(xs)
    if len(xs) == 0:
        return None
    breaks = np.where(np.diff(xs) != 1)[0]
    starts_i = np.concatenate([[0], breaks + 1])
    ends_i = np.concatenate([breaks, [len(xs) - 1]])
    run_starts = xs[starts_i]
    run_lens = ends_i - starts_i + 1
    if not np.all(run_lens == run_lens[0]):
        return None
    L = int(run_lens[0])
    if len(run_starts) == 1:
        return (int(run_starts[0]), L, 0, 1, 0, 1)
    d = np.diff(run_starts)
    if np.all(d == d[0]):
        return (int(run_starts[0]), L, int(d[0]), len(run_starts), 0, 1)
    s1 = d[0]
    c1 = 1
    while c1 < len(d) and d[c1 - 1] == s1:
        c1 += 1
    group = c1
    if len(run_starts) % group != 0:
        return None
    rs = run_starts.reshape(-1, group)
    inner = np.diff(rs, axis=1)
    starts2 = rs[:, 0]
    d2 = np.diff(starts2)
    if inner.size and not np.all(inner == s1):
        return None
    if len(d2) and not np.all(d2 == d2[0]):
        return None
    return (int(run_starts[0]), L, int(s1), group,
            int(d2[0]) if len(d2) else 0, len(starts2))


def emit_ops():
    """Returns list of (p, k, [ops]); op = (r0, nrows, drow, colpat, colB0)
    colpat=(c0,L,s1,c1,s2,c2) for A; B cols = A cols + (colB0 - c0)."""
    all_stages = []
    for (p, k, lefts) in stages():
        ops = []
        rows = lefts // NCOLS
        cols = lefts % NCOLS
        drows = (lefts + k) // NCOLS - rows
        for dr in np.unique(drows):
            sel = drows == dr
            rset = np.unique(rows[sel])
            cset = np.unique(cols[sel])
            # must be Cartesian product
            assert sel.sum() == len(rset) * len(cset), (p, k, dr)
            for r in rset:
                cc = np.sort(cols[sel & (rows == r)])
                assert np.array_equal(cc, cset), (p, k, dr, r)
            colpat = runs_of(cset)
            assert colpat is not None, (p, k, dr, cset[:20])
            # split rset into contiguous runs
            rpat = runs_of(rset)
            assert rpat is not None, (p, k, dr, rset)
            (r0, Lr, sr1, cr1, sr2, cr2) = rpat
            assert sr2 == 0 and cr2 == 1, (p, k, dr, rpat)
            colB0 = int((cset[0] + k) % NCOLS)
            for g in range(cr1):
                rstart = r0 + g * sr1
                ops.append((int(rstart), int(Lr), int(dr), colpat, colB0))
        all_stages.append((p, int(k), ops))
    return all_stages


def _row_chunks(a_base, b_base, nr):
    """Split nr rows so every chunk's A and B partition ranges are legal:
    base 0 -> <=4 rows (128 parts), 32 -> 1, 64 -> 2, 96 -> 1 (32*r bases)."""
    allowed = {0: 4, 1: 1, 2: 2, 3: 1}
    out = []
    off = 0
    while off < nr:
        c = min(allowed[(a_base + off) % 4], allowed[(b_base + off) % 4], nr - off)
        out.append((off, c))
        off += c
    return out


def legalize(all_stages):
    """Split op-groups to satisfy partition-base rules."""
    out = []
    for (p, k, ops) in all_stages:
        nops = []
        for (r0, nr, dr, colpat, colB0) in ops:
            for (off, c) in _row_chunks(r0, r0 + dr, nr):
                nops.append((r0 + off, c, dr, colpat, colB0))
        out.append((p, k, nops))
    return out


def colpat_idx(colpat):
    (c0, L, s1, c1, s2, c2) = colpat
    return (c0 + np.arange(c2)[:, None, None] * s2
            + np.arange(c1)[None, :, None] * s1
            + np.arange(L)[None, None, :]).ravel()


def simulate(all_stages, arr):
    a = arr.copy()
    for (p, k, ops) in all_stages:
        for (r0, nr, dr, colpat, colB0) in ops:
            ia = colpat_idx(colpat)
            ib = ia + (colB0 - colpat[0])
            A = a[r0:r0 + nr, ia]
            Bm = a[r0 + dr:r0 + dr + nr, ib]
            mn = np.minimum(A, Bm)
            mx = np.maximum(A, Bm)
            a[r0:r0 + nr, ia] = mn
            a[r0 + dr:r0 + dr + nr, ib] = mx
    return a


def dve_cycles(all_stages):
    tot = 0.0
    n_ops = 0
    for (p, k, ops) in all_stages:
        for (r0, nr, dr, colpat, colB0) in ops:
            free = colpat[1] * colpat[3] * colpat[5]
            tot += (58 + free) * 2 + (58 + free / 2)
            n_ops += 3
    return tot, n_ops


def runs_multi(xs, max_groups=6):
    """Compress sorted ints into a list of <=3-level patterns.
    Groups runs by run-length first (complements of periodic patterns are
    unions of uniform-run periodic sets)."""
    xs = np.asarray(xs)
    if len(xs) == 0:
        return []
    r = runs_of(xs)
    if r is not None:
        return [r]
    breaks = np.where(np.diff(xs) != 1)[0]
    starts_i = np.concatenate([[0], breaks + 1])
    ends_i = np.concatenate([breaks, [len(xs) - 1]])
    run_starts = xs[starts_i]
    run_lens = ends_i - starts_i + 1
    out = []
    for L in np.unique(run_lens):
        sel = run_lens == L
        rs = run_starts[sel]
        # each group: runs of identical length -> starts should be periodic
        d = np.diff(rs)
        if len(d) == 0 or np.all(d == d[0]):
            out.append((int(rs[0]), int(L), int(d[0]) if len(d) else 0,
                        len(rs), 0, 1))
        else:
            # fall back: one op per run
            for s in rs:
                out.append((int(s), int(L), 0, 1, 0, 1))
    return out


def emit_pingpong():
    """Stages with compare ops + complement copy ops for ping-pong buffers.
    Returns list of (p, k, cmp_ops, cp_ops):
      cmp op: (r0, nr, dr, colpat, colB0)
      cp op:  (r0, nr, colpat)
    """
    out = []
    for (p, k, ops) in legalize(emit_ops()):
        touched = np.zeros((NROWS, NCOLS), dtype=bool)
        for (r0, nr, dr, colpat, colB0) in ops:
            ia = colpat_idx(colpat)
            ib = ia + (colB0 - colpat[0])
            for rr in range(r0, r0 + nr):
                touched[rr, ia] = True
                touched[rr + dr, ib] = True
        cp_ops = []
        # group contiguous rows with identical complement masks
        r = 0
        while r < NROWS:
            mask = ~touched[r]
            r2 = r + 1
            while r2 < NROWS and np.array_equal(~touched[r2], mask):
                r2 += 1
            cols = np.where(mask)[0]
            if len(cols):
                for pat in runs_multi(cols):
                    # legal row chunks for 1-input ops (A base only)
                    off = 0
                    nr_ = r2 - r
                    allowed = {0: 4, 1: 1, 2: 2, 3: 1}
                    while off < nr_:
                        c = min(allowed[(r + off) % 4], nr_ - off)
                        cp_ops.append((r + off, c, pat))
                        off += c
            r = r2
        out.append((p, k, ops, cp_ops))
    return out


def simulate_pp(stages_pp, arr):
    """Ping-pong simulation: validates full coverage each stage."""
    cur = arr.copy()
    for (p, k, cmp_ops, cp_ops) in stages_pp:
        nxt = np.full_like(cur, np.nan)
        for (r0, nr, dr, colpat, colB0) in cmp_ops:
            ia = colpat_idx(colpat)
            ib = ia + (colB0 - colpat[0])
            A = cur[r0:r0 + nr, ia]
            Bm = cur[r0 + dr:r0 + dr + nr, ib]
            nxt[r0:r0 + nr, ia] = np.minimum(A, Bm)
            nxt[r0 + dr:r0 + dr + nr, ib] = np.maximum(A, Bm)
        for (r0, nr, pat) in cp_ops:
            ic = colpat_idx(pat)
            nxt[r0:r0 + nr, ic] = cur[r0:r0 + nr, ic]
        assert not np.isnan(nxt).any(), (p, k, "coverage hole")
        cur = nxt
    return cur


if __name__ == "__main__" or True:
    pass


def _split_colpat(colpat, max_free=288):
    """Split a colpat into pieces each with free size <= max_free.
    Returns list of (delta_offset, colpat)."""
    (c0, L, s1, c1, s2, c2) = colpat
    free = L * c1 * c2
    if free <= max_free:
        return [(0, colpat)]
    if c2 > 1:
        h = c2 // 2
        a = (c0, L, s1, c1, s2, h)
        b = (c0 + h * s2, L, s1, c1, s2, c2 - h)
        return [(d, p) for d0, pp_ in [(0, a), (h * s2, b)]
                for d, p in [(d0 + dd, p2) for dd, p2 in _split_colpat(
                    (pp_[0], pp_[1], pp_[2], pp_[3], pp_[4], pp_[5]), max_free)]]
    if c1 > 1:
        h = c1 // 2
        a = (c0, L, s1, h, 0, 1)
        b = (c0 + h * s1, L, s1, c1 - h, 0, 1)
        out = []
        for base, pat in [(0, a), (h * s1, b)]:
            out.extend(_split_colpat(pat, max_free))
        return out
    h = L // 2
    a = (c0, h, 0, 1, 0, 1)
    b = (c0 + h, L - h, 0, 1, 0, 1)
    return _split_colpat(a, max_free) + _split_colpat(b, max_free)


def drain_split(stages_pp, max_free=288):
    """Split big cmp/copy ops so DVE drain overhead stays bounded."""
    out = []
    for (p, k, cmp_ops, cp_ops) in stages_pp:
        nc_ops = []
        for (r0, nr, dr, colpat, colB0) in cmp_ops:
            for (_, pat) in _split_colpat(colpat, max_free):
                nb0 = colB0 + (pat[0] - colpat[0])
                nc_ops.append((r0, nr, dr, pat, nb0))
        ncp_ops = []
        for (r0, nr, pat) in cp_ops:
            for (_, p2) in _split_colpat(pat, max_free):
                ncp_ops.append((r0, nr, p2))
        out.append((p, k, nc_ops, ncp_ops))
    return out


def gen_pingpong(n, nrows, ncols, p_min=1, max_free=288):
    """Parametric ping-pong network for n = nrows*ncols fold, phases p >= p_min."""
    global N, NROWS, NCOLS
    oldN, oldR, oldC = N, NROWS, NCOLS
    N, NROWS, NCOLS = n, nrows, ncols
    try:
        full = emit_pingpong()
        filt = [(p, k, c, cp) for (p, k, c, cp) in full if p >= p_min]
        return drain_split(filt, max_free)
    finally:
        N, NROWS, NCOLS = oldN, oldR, oldC



import bass_rust
import concourse.bacc as bacc
import concourse.mybir as mybir
from concourse import tile
from concourse.bass_utils import run_bass_kernel_spmd
from concourse.masks import make_identity


B, D, H1, H2, F = 2048, 3072, 512, 256, 100
NCORES = 8
# packed fp32 constant columns: iotas | iotac | b1r | b2r | wcol | bf
C_IOTC, C_B1, C_B2, C_WC, C_BF = 256, 768, 772, 774, 775
CW = 776
BS = B // NCORES            # 256 rows per core
LEAK = 0.2
P = 128
FL = 13                     # features per core (8*13 = 104 >= 100)
FPAD = NCORES * FL          # 104
NR, NC = NROWS, NCOLS   # 4, 512
RC = 2.0 ** 23              # rounding constant
QLEV = 8190.0               # quantization levels (margin below 2^13)
MRANGE = 16.0               # fixed m-quantization range [-16, 16)
QSCALE = QLEV / (2 * MRANGE)
DQ = (2 * MRANGE) / QLEV
FSCALE = QLEV / 2048.0
FDEC = 2048.0 / QLEV

f32 = mybir.dt.float32
AF = mybir.ActivationFunctionType
ALU = mybir.AluOpType

def mkap(t_ap, dims, offset):
    """Arbitrary strided AP view: dims = [(stride, count), ...]."""
    a = t_ap.copy()
    a.ap = bass_rust.VecI64Pair([tuple(d) for d in dims])
    a.offset = offset
    return a


def sap(t_ap, pitch, pstart, pcount, coff, colpat):
    """Build a strided AP view: partitions [pstart, pstart+pcount), free
    pattern colpat=(c0,L,s1,c1,s2,c2) shifted to coff."""
    (c0, L, s1, c1, s2, c2) = colpat
    dims = [(pitch, pcount)]
    if c2 > 1:
        dims.append((s2, c2))
    if c1 > 1:
        dims.append((s1, c1))
    dims.append((1, L))
    a = t_ap.copy()
    a.ap = bass_rust.VecI64Pair(dims)
    a.offset = pstart * pitch + coff
    return a


SRC_OPS = gen_pingpong(256, 1, 256)
MRG_OPS = gen_pingpong(2048, 4, 512, p_min=256, max_free=10 ** 9)


def _split_ratio(colpat, n1_frac):
    """Split colpat into two pieces along the outermost multi-count dim,
    with roughly n1_frac of the free size in the first piece.
    Returns [(colpatA), (colpatB)] (col offsets already absolute)."""
    (c0, L, s1, c1, s2, c2) = colpat
    if c2 > 1:
        h = min(max(1, int(round(c2 * n1_frac))), c2 - 1)
        return [(c0, L, s1, c1, s2, h),
                (c0 + h * s2, L, s1, c1, s2, c2 - h)]
    if c1 > 1:
        h = min(max(1, int(round(c1 * n1_frac))), c1 - 1)
        return [(c0, L, s1, h, 0, 1), (c0 + h * s1, L, s1, c1 - h, 0, 1)]
    h = min(max(1, int(round(L * n1_frac))), L - 1)
    return [(c0, h, 0, 1, 0, 1), (c0 + h, L - h, 0, 1, 0, 1)]


# per-engine cost models (ns) for scheduling decisions
def _c_dve_tt(f):
    return 200 + 1.108 * f


def _c_dve_cp(f):
    return 200 + 0.554 * f


def _c_pool(f):
    return 345 + 1.36 * f


def _c_act_cp(f):
    return 230 + 1.33 * f


def emit_sort_sched(nc, ops_table, rowpart, buf, tmp, mir_tiles, pitch,
                    split_min=320):
    """Ping-pong odd-even mergesort with cost-balanced engine assignment.
    Compare min/max ops go to DVE/Pool (large ones split by columns);
    complement copies and B-operand mirrors go to DVE/Pool/ACT.
    mir_tiles: list of SBUF scratch tiles [128, NCOLS] for mirrors."""
    bufs = [buf, tmp]
    mi = 0

    def emit_cmp(eng, cur, nxt, r0, nr, dr, pat, b0, mir_ap, which):
        pa, pb = rowpart * r0, rowpart * (r0 + dr)
        npart = rowpart * nr
        a_in = sap(cur, pitch, pa, npart, pat[0], pat)
        b_in = (sap(cur, pitch, pb, npart, b0, pat) if dr == 0
                else sap(mir_ap, pitch, pa, npart, pat[0], pat))
        if which == 0:
            o = sap(nxt, pitch, pa, npart, pat[0], pat)
            alu = ALU.min
        else:
            o = sap(nxt, pitch, pb, npart, b0, pat)
            alu = ALU.max
        if eng == "DVE":
            nc.vector.tensor_tensor(o, a_in, b_in, alu)
        else:
            nc.gpsimd.tensor_tensor(o, a_in, b_in, alu)

    for si, (p, k, cmp_ops, cp_ops) in enumerate(ops_table):
        cur = bufs[si % 2]
        nxt = bufs[(si + 1) % 2]
        mirrors = []   # (cost_fns, emit_fn args)
        works = []
        for (r0, nr, dr, colpat, colB0) in cmp_ops:
            free = colpat[1] * colpat[3] * colpat[5]
            if dr != 0:
                mt = mir_tiles[mi % len(mir_tiles)]
                mi += 1
                mir_ap = mt[:]
                pa, pb = rowpart * r0, rowpart * (r0 + dr)
                npart = rowpart * nr
                # mirror B to A's partition base (SBUF scratch)
                mirrors.append((free, "cp",
                                (sap(mir_ap, pitch, pa, npart, colpat[0],
                                     colpat),
                                 sap(cur, pitch, pb, npart, colB0, colpat))))
            else:
                mir_ap = None
            # compares are DVE-only: splitting them only adds overhead
            for (pat, b0) in [(colpat, colB0)]:
                pf = pat[1] * pat[3] * pat[5]
                for which in (0, 1):
                    works.append((pf, "cmp",
                                  (cur, nxt, r0, nr, dr, pat, b0, mir_ap,
                                   which)))
        for (r0, nr, pat) in cp_ops:
            free = pat[1] * pat[3] * pat[5]
            pats = [pat] if free < split_min else _split_ratio(pat, 0.5)
            for pp_ in pats:
                pf = pp_[1] * pp_[3] * pp_[5]
                pa = rowpart * r0
                npart = rowpart * nr
                works.append((pf, "cp",
                              (sap(nxt, pitch, pa, npart, pp_[0], pp_),
                               sap(cur, pitch, pa, npart, pp_[0], pp_))))

        loads = {"DVE": 0.0, "POOL": 0.0, "ACT": 0.0}

        def place(f, kind):
            if kind == "cmp":
                # Pool has no TensorTensor opcode on HW: compares are DVE-only
                cand = {"DVE": _c_dve_tt(f)}
            else:
                cand = {"POOL": _c_pool(f), "ACT": _c_act_cp(f)}
            e = min(cand, key=lambda x: loads[x] + cand[x])
            loads[e] += cand[e]
            return e

        # mirrors first (they gate compares)
        for (f, kind, (o, i_)) in sorted(mirrors, key=lambda w: -w[0]):
            e = place(f, "cp")
            if e == "DVE":
                nc.vector.tensor_copy(o, i_)
            elif e == "POOL":
                nc.gpsimd.tensor_copy(o, i_)
            else:
                nc.scalar.copy(o, i_)
        for (f, kind, args) in sorted(works, key=lambda w: -w[0]):
            e = place(f, kind)
            if kind == "cmp":
                emit_cmp(e, *args)
            else:
                (o, i_) = args
                if e == "DVE":
                    nc.vector.tensor_copy(o, i_)
                elif e == "POOL":
                    nc.gpsimd.tensor_copy(o, i_)
                else:
                    nc.scalar.copy(o, i_)


def emit_sort(nc, ops_table, rowpart, buf, tmp, pmirror, pitch, ppitch,
              mirror_copy, cp_engines, mir_pool=None):
    """Ping-pong odd-even mergesort between `buf` and `tmp` ([128, NC] tiles).
    Each stage: DVE min/max write the other buffer; untouched cells are
    copied across by cp_engines (ACT/GPSIMD); cross-row compares read the
    B operand through a PSUM mirror (mirror_copy must reach PSUM).
    len(SORT_PP) is even, so the result lands back in `buf`."""
    bufs = [buf, tmp]
    ci = 0
    for si, (p, k, cmp_ops, cp_ops) in enumerate(ops_table):
        cur = bufs[si % 2]
        nxt = bufs[(si + 1) % 2]
        for (r0, nr, dr, colpat, colB0) in cmp_ops:
            pa, pb = rowpart * r0, rowpart * (r0 + dr)
            npart = rowpart * nr
            a_in = sap(cur, pitch, pa, npart, colpat[0], colpat)
            a_out = sap(nxt, pitch, pa, npart, colpat[0], colpat)
            b_out = sap(nxt, pitch, pb, npart, colB0, colpat)
            if dr == 0:
                b_in = sap(cur, pitch, pb, npart, colB0, colpat)
            else:
                b_cur = sap(cur, pitch, pb, npart, colB0, colpat)
                if mir_pool is not None:
                    mt = mir_pool.tile([128, 512], mybir.dt.float32,
                                       tag="mir", bufs=6, name="mirt")
                    b_in = sap(mt[:], mt[:].ap[0][0], pa, npart,
                               colpat[0], colpat)
                else:
                    b_in = sap(pmirror, ppitch, pa, npart, colpat[0], colpat)
                mirror_copy(b_in, b_cur)
            nc.vector.tensor_tensor(a_out, a_in, b_in, ALU.min)
            nc.vector.tensor_tensor(b_out, a_in, b_in, ALU.max)
        for (r0, nr, pat) in cp_ops:
            pa = rowpart * r0
            npart = rowpart * nr
            c_in = sap(cur, pitch, pa, npart, pat[0], pat)
            c_out = sap(nxt, pitch, pa, npart, pat[0], pat)
            cp_engines[ci % len(cp_engines)](c_out, c_in)
            ci += 1


def build_program(dbg=False, repeat=1, upto=99):
    nc = bacc.Bacc(
        "TRN2", target_bir_lowering=False, debug=False, num_devices=NCORES)

    f16 = mybir.dt.float16
    xT = nc.dram_tensor("xT", [D, BS], f16, kind="ExternalInput").ap()
    W1 = nc.dram_tensor("W1", [D, H1], f16, kind="ExternalInput").ap()
    W2 = nc.dram_tensor("W2", [H1, H2], f16, kind="ExternalInput").ap()
    Tm = nc.dram_tensor("Tm", [H2, F], f16, kind="ExternalInput").ap()
    Wfh = nc.dram_tensor("Wfh", [H2, 1], f16, kind="ExternalInput").ap()
    # packed fp32 constants: iotas | iotac | b1r | b2r | wcol | bf
    cst = nc.dram_tensor("cst", [P, CW], f32, kind="ExternalInput").ap()
    out = nc.dram_tensor("out", [1, B], f32, kind="ExternalOutput").ap()
    outh = nc.dram_tensor("outh", [1, BS], f32, kind="ExternalOutput").ap()

    dbg_aps = {}
    if dbg:
        for nm, shp in [("d_msort", [P, NC]), ("d_key", [P, NC]),
                        ("d_sorted", [P, NC]), ("d_u", [P, NC]),
                        ("d_s1u", [P, NC]), ("d_s2v", [P, NC]),
                        ("d_feats", [P, NC]), ("d_key2s", [P, NC]),
                        ("d_fdec", [P, NC]), ("d_scal", [P, 8]),
                        ("d_contrib", [1, B]), ("d_f13", [FL, B])]:
            dbg_aps[nm] = nc.dram_tensor(nm, shp, f32, kind="ExternalOutput").ap()

    KD, K1, K2 = D // P, H1 // P, H2 // P

    with tile.TileContext(nc) as tc:
      for _rep in range(repeat):
        with (
            tc.tile_pool(name="persist", bufs=1) as pers,
            tc.tile_pool(name="dram", bufs=1, space="DRAM") as dpool,
        ):
            # ======== persistent tiles (DMAs issued after big streams) ====
            cst_sb = pers.tile([P, CW], f32)
            Wcol_sb = pers.tile([P, 1], mybir.dt.float16)
            hWf_sb = pers.tile([1, BS], f32)
            mT_loc = pers.tile([F, BS], f32)
            key = pers.tile([P, NC], f32)
            kpitch = key[:].ap[0][0]

            # ======== phase 1: MLP ========
            with (
                tc.tile_pool(name="mlp", bufs=1) as mp,
                tc.tile_pool(name="psum_mm", bufs=1, space="PSUM") as pmm,
            ):
                # grouped streaming loads: few big DMAs (HWDGE descriptor
                # generation serializes per dma_start), PE consumes each
                # k-group as it lands
                GRP = 4
                KG = KD // GRP
                xTall = mp.tile([P, KD * BS], f16, name="xTall")
                W1all = mp.tile([P, KD * H1], f16, name="W1all")
                W2all = mp.tile([P, K1 * H2], f16, name="W2all")
                Tall = mp.tile([P, K2 * F], f16, name="Tall")
                Wfh_sb = mp.tile([P, K2], f16)
                xpitch = xTall[:].ap[0][0]
                w1pitch = W1all[:].ap[0][0]
                for g in range(GRP):
                    rs = g * KG * P
                    nc.sync.dma_start(
                        mkap(xTall[:], [[xpitch, P], [BS, KG], [1, BS]],
                             g * KG * BS),
                        mkap(xT, [[BS, P], [P * BS, KG], [1, BS]], rs * BS))
                    nc.sync.dma_start(
                        mkap(W1all[:], [[w1pitch, P], [H1, KG], [1, H1]],
                             g * KG * H1),
                        mkap(W1, [[H1, P], [P * H1, KG], [1, H1]], rs * H1))
                nc.sync.dma_start(
                    mkap(W2all[:], [[W2all[:].ap[0][0], P], [H2, K1], [1, H2]],
                         0),
                    mkap(W2, [[H2, P], [P * H2, K1], [1, H2]], 0))
                nc.sync.dma_start(
                    mkap(Tall[:], [[Tall[:].ap[0][0], P], [F, K2], [1, F]], 0),
                    mkap(Tm, [[F, P], [P * F, K2], [1, F]], 0))
                nc.sync.dma_start(
                    Wfh_sb[:], Wfh.rearrange("(k p) one -> p (k one)", p=P))
                # constants + junk feature rows (row-major sorted values:
                # decode to distinct harmless scatter indices and are
                # merge-invariant) — queued behind the big streams
                nc.sync.dma_start(cst_sb[:], cst)
                nc.vector.tensor_copy(Wcol_sb[:], cst_sb[:, C_WC:C_WC + 1])
                for r in range(NR):
                    nc.sync.dma_start(
                        key[32 * r + FL:32 * (r + 1), :],
                        mkap(cst, [[CW, 32 - FL], [1, NC]],
                             (32 * r + FL) * CW + C_IOTC))

                h1T = [mp.tile([P, BS], f16, name=f"h1T{m}") for m in range(K1)]
                pt1 = [pmm.tile([P, BS], f32, tag=f"mm{m}", name=f"pt1_{m}")
                       for m in range(K1)]
                for k in range(KD):
                    for mb in range(K1):
                        nc.tensor.matmul(
                            pt1[mb][:],
                            W1all[:, k * H1 + mb * P:k * H1 + (mb + 1) * P],
                            xTall[:, k * BS:(k + 1) * BS],
                            start=(k == 0), stop=(k == KD - 1))
                for mb in range(K1):
                    s1 = mp.tile([P, BS], f32, tag="stmp", bufs=2, name=f"s1_{mb}")
                    nc.scalar.activation(
                        s1[:], pt1[mb][:], AF.Identity,
                        bias=cst_sb[:, C_B1 + mb:C_B1 + mb + 1])
                    nc.vector.scalar_tensor_tensor(
                        h1T[mb][:], s1[:], LEAK, s1[:], op0=ALU.mult, op1=ALU.max)

                h2T = [mp.tile([P, BS], f16, name=f"h2T{m}") for m in range(K2)]
                for mb in range(K2):
                    pt = pmm.tile([P, BS], f32, tag=f"mm2_{mb}",
                                  name=f"pt2_{mb}")
                    for k in range(K1):
                        nc.tensor.matmul(
                            pt[:],
                            W2all[:, k * H2 + mb * P:k * H2 + (mb + 1) * P],
                            h1T[k][:],
                            start=(k == 0), stop=(k == K1 - 1))
                    s2 = mp.tile([P, BS], f32, tag="stmp", bufs=2, name=f"s2_{mb}")
                    nc.scalar.activation(
                        s2[:], pt[:], AF.Identity,
                        bias=cst_sb[:, C_B2 + mb:C_B2 + mb + 1])
                    nc.vector.scalar_tensor_tensor(
                        h2T[mb][:], s2[:], LEAK, s2[:], op0=ALU.mult, op1=ALU.max)

                pt_m = pmm.tile([F, BS], f32, tag="mm")
                for k in range(K2):
                    nc.tensor.matmul(
                        pt_m[:], Tall[:, k * F:(k + 1) * F], h2T[k][:],
                        start=(k == 0), stop=(k == K2 - 1))
                nc.scalar.copy(mT_loc[:], pt_m[:])

                ph = pmm.tile([1, BS], f32, tag="hw")
                for k in range(K2):
                    nc.tensor.matmul(
                        ph[:], Wfh_sb[:, k:k + 1], h2T[k][:],
                        start=(k == 0), stop=(k == K2 - 1))
                nc.vector.tensor_copy(hWf_sb[:], ph[:])

            if upto <= 1:
                nc.sync.dma_start(out[:, 0:BS], mT_loc[0:1, :])
                continue
            # ======== phase 2: AllToAll #1 ========
            skey = pers.tile([P, BS], f32)
            sktmp = pers.tile([P, BS], f32)
            nc.vector.memset(skey[:], 0.0)
            nc.vector.tensor_scalar(
                skey[:F, :], mT_loc[:], scalar1=MRANGE, scalar2=QSCALE,
                op0=ALU.add, op1=ALU.mult)
            nc.vector.tensor_scalar(
                skey[:F, :], skey[:F, :], scalar1=RC, scalar2=RC,
                op0=ALU.add, op1=ALU.subtract)
            nc.vector.tensor_scalar(
                skey[:F, :], skey[:F, :], scalar1=8191.0, scalar2=0.0,
                op0=ALU.min, op1=ALU.max)
            nc.vector.tensor_tensor(skey[:F, :], skey[:F, :],
                                    cst_sb[:F, 0:BS], ALU.add)
            spitch = skey[:].ap[0][0]
            emit_sort_sched(nc, SRC_OPS, P, skey[:], sktmp[:], [], spitch)
            if upto <= 2:
                nc.sync.dma_start(out[:, 0:BS], skey[0:1, 0:BS])
                continue
            a2a_in = dpool.tile([FPAD, BS], f32)
            a2a_out = dpool.tile([FPAD, BS], f32)
            nc.sync.dma_start(a2a_in[:F, :], skey[:F, :])
            # rows 100:104 = copies of features 0:4 (benign padding)
            nc.sync.dma_start(a2a_in[F:FPAD, :], skey[:FPAD - F, :])
            nc.gpsimd.collective_compute(
                "AllToAll", ALU.bypass,
                replica_groups=[list(range(NCORES))],
                ins=[a2a_in.opt()], outs=[a2a_out.opt()])

            # a2a result -> quadrant layout: one DMA per quadrant row
            # (SBUF side needs a single partition range; DRAM side is 3-dim)
            for r in range(NR):
                nc.sync.dma_start(
                    mkap(key[:], [[kpitch, FL], [1, 2 * BS]], 32 * r * kpitch),
                    mkap(a2a_out[:], [[BS, FL], [FL * BS, 2], [1, BS]],
                         2 * r * FL * BS))

            # ======== phase 3 ========
            pitch = key[:].ap[0][0]
            with (
                tc.tile_pool(name="sortp", bufs=1) as sp,
                tc.tile_pool(name="psum2", bufs=1, space="PSUM") as pp2,
            ):
                # ======== phase 4: merge (30 stages) ========
                tmp = sp.tile([P, NC], f32)
                pmir = pp2.tile([P, NC], f32, tag="mir", bufs=1, name="pmir")
                mir_sb = [sp.tile([P, NC], f32, name=f"mir{i}")
                          for i in range(4)]
                emit_sort_sched(nc, MRG_OPS, 32, key[:], tmp[:], mir_sb,
                                pitch)
                if dbg:
                    nc.sync.dma_start(dbg_aps["d_sorted"][:], key[:])

                if upto <= 4:
                    nc.sync.dma_start(out[:, 0:BS], key[0:1, 0:BS])
                    continue
                # ======== phase 5: feats in sorted order ========
                # split key = g + j/2048 via integer masking of key*2048
                ki = sp.tile([P, NC], mybir.dt.int32)
                kq = sp.tile([P, NC], f32)
                nc.vector.tensor_scalar_mul(kq[:], key[:], 2048.0)
                nc.vector.tensor_copy(ki[:], kq[:])
                # scatter indices (arith-only, no bitwise+arith fusion);
                # one 1024-wide scatter per destination row PAIR:
                #  rows 0/1: 1023-j (reversed; j>=1024 -> negative)
                #  rows 2/3: j-1024 (j<1024 -> negative)
                j32 = sp.tile([P, NC], mybir.dt.int32, name="j32")
                nc.vector.tensor_scalar(
                    j32[:], ki[:], scalar1=2047, scalar2=None,
                    op0=ALU.bitwise_and)
                idx16 = []
                for g, (mul, add) in enumerate([(-1, 1023), (1, -1024)]):
                    ix = sp.tile([P, NC], mybir.dt.int16, name=f"ix{g}")
                    nc.vector.tensor_scalar(
                        ix[:], j32[:], scalar1=mul, scalar2=add,
                        op0=ALU.mult, op1=ALU.add)
                    idx16.append(ix)
                gi = sp.tile([P, NC], mybir.dt.int32)
                nc.vector.tensor_scalar(
                    gi[:], ki[:], scalar1=-2048, scalar2=None, op0=ALU.bitwise_and)
                g2k = sp.tile([P, NC], f32)   # g * 2048
                nc.vector.tensor_copy(g2k[:], gi[:])
                bneg = sp.tile([P, 1], f32)
                nc.vector.memset(bneg[:], -MRANGE)
                bpos = sp.tile([P, 1], f32)
                nc.vector.memset(bpos[:], MRANGE)
                u = sp.tile([P, NC], f32)
                nc.scalar.activation(
                    u[:], g2k[:], AF.Exp, bias=bneg[:], scale=DQ / 2048.0)
                v = sp.tile([P, NC], f32)
                nc.scalar.activation(
                    v[:], g2k[:], AF.Exp, bias=bpos[:], scale=-DQ / 2048.0)

                su = sp.tile([P, NC], f32)
                nc.vector.tensor_tensor_scan(
                    su[:], u[:], u[:], initial=0.0, op0=ALU.add, op1=ALU.bypass)
                sv = sp.tile([P, NC], f32)
                nc.vector.tensor_tensor_scan(
                    sv[:, NC - 1::-1], v[:, NC - 1::-1], v[:, NC - 1::-1],
                    initial=0.0, op0=ALU.add, op1=ALU.bypass)

                # cross-row carries (prefix over 4 quadrant rows)
                def shift_add(dst, src_lo, src_hi, bounce):
                    # dst[hi] += dst[lo] via PSUM bounce (base-change copy)
                    nc.vector.tensor_copy(bounce[src_hi[0]:src_hi[1], :],
                                          dst[src_lo[0]:src_lo[1], :])
                    nc.vector.tensor_tensor(
                        dst[src_hi[0]:src_hi[1], :],
                        dst[src_hi[0]:src_hi[1], :],
                        bounce[src_hi[0]:src_hi[1], :], ALU.add)

                cu = sp.tile([P, 2], f32)   # col0: inclusive row totals
                nc.vector.tensor_copy(cu[:, 0:1], su[:, NC - 1:NC])
                bu = pmir[:, 0:1]
                shift_add(cu[:, 0:1], (0, 32), (32, 64), bu)
                shift_add(cu[:, 0:1], (32, 64), (64, 96), bu)
                shift_add(cu[:, 0:1], (64, 96), (96, 128), bu)
                nc.vector.tensor_tensor(
                    cu[:, 1:2], cu[:, 0:1], su[:, NC - 1:NC], ALU.subtract)
                cv = sp.tile([P, 2], f32)   # suffix carries (from higher rows)
                nc.vector.tensor_copy(cv[:, 0:1], sv[:, 0:1])
                bv = pmir[:, 1:2]
                shift_add(cv[:, 0:1], (96, 128), (64, 96), bv)
                shift_add(cv[:, 0:1], (64, 96), (32, 64), bv)
                shift_add(cv[:, 0:1], (32, 64), (0, 32), bv)
                nc.vector.tensor_tensor(
                    cv[:, 1:2], cv[:, 0:1], sv[:, 0:1], ALU.subtract)

                s1u = sp.tile([P, NC], f32)
                nc.vector.tensor_scalar(
                    s1u[:], su[:], scalar1=cu[:, 1:2], scalar2=None, op0=ALU.add)
                s2vi = sp.tile([P, NC], f32)
                nc.vector.tensor_scalar(
                    s2vi[:], sv[:], scalar1=cv[:, 1:2], scalar2=None, op0=ALU.add)
                nc.vector.tensor_tensor(s2vi[:], s2vi[:], v[:], ALU.subtract)

                feats = sp.tile([P, NC], f32)
                nc.vector.tensor_tensor(feats[:], v[:], s1u[:], ALU.mult)
                fb = sp.tile([P, NC], f32)
                nc.vector.tensor_tensor(fb[:], u[:], s2vi[:], ALU.mult)
                nc.vector.tensor_tensor(feats[:], feats[:], fb[:], ALU.add)
                if dbg:
                    nc.sync.dma_start(dbg_aps["d_u"][:], u[:])
                    nc.sync.dma_start(dbg_aps["d_s1u"][:], s1u[:])
                    nc.sync.dma_start(dbg_aps["d_s2v"][:], s2vi[:])
                    nc.sync.dma_start(dbg_aps["d_feats"][:], feats[:])

                if upto <= 5:
                    nc.sync.dma_start(out[:, 0:BS], feats[0:1, 0:BS])
                    continue
                # ======== phase 6': scatter feats to original order ========
                # per destination row r2: within-partition scatter by index
                # (GPSIMD), then one matmul folds the cross-row reduction and
                # the Wf contraction: contrib[c] = sum_p Wcol[p]*buf[p, c].
                fh = sp.tile([P, NC], mybir.dt.float16, name="fh")
                nc.vector.tensor_copy(fh[:], feats[:])
                contrib = sp.tile([1, B], f32)
                sbufs = []
                for g in range(2):
                    buf = sp.tile([P, 2 * NC], mybir.dt.float16,
                                  name=f"scb{g}")
                    nc.gpsimd.local_scatter(buf[:], fh[:], idx16[g][:],
                                            P, 2 * NC, NC)
                    sbufs.append(buf)
                for r2 in range(NR):
                    pcon = pp2.tile([1, NC], f32, tag="con", bufs=2,
                                    name=f"pcon{r2}")
                    buf = sbufs[r2 // 2]
                    if r2 == 0:        # c' = 1023-j, j in [0,511]
                        mov = buf[:, 2 * NC - 1:NC - 1:-1]
                    elif r2 == 1:      # c' = 1023-j, j in [512,1023]
                        mov = buf[:, NC - 1::-1]
                    elif r2 == 2:      # c' = j-1024
                        mov = buf[:, 0:NC]
                    else:
                        mov = buf[:, NC:2 * NC]
                    nc.tensor.matmul(pcon[:], Wcol_sb[:], mov,
                                     start=True, stop=True)
                    if r2 % 2 == 0:
                        nc.vector.tensor_copy(
                            contrib[:, r2 * NC:(r2 + 1) * NC], pcon[:])
                    else:
                        nc.scalar.copy(
                            contrib[:, r2 * NC:(r2 + 1) * NC], pcon[:])
                nc.sync.dma_start(out[:], contrib[:])
                osb = sp.tile([1, BS], f32)
                nc.vector.tensor_scalar(
                    osb[:], hWf_sb[:], scalar1=cst_sb[0:1, C_BF:C_BF + 1],
                    scalar2=None,
                    op0=ALU.add)
                nc.sync.dma_start(outh[:], osb[:])

    nc.compile()
    return nc


def _build_in_maps(inputs):
    x = np.asarray(inputs["x"], np.float32)
    W1 = np.asarray(inputs["W1"], np.float32)
    b1 = np.asarray(inputs["b1"], np.float32)
    W2 = np.asarray(inputs["W2"], np.float32)
    b2 = np.asarray(inputs["b2"], np.float32)
    T = np.asarray(inputs["T"], np.float32)
    Wf = np.asarray(inputs["Wf"], np.float32)
    bf = np.asarray(inputs["bf"], np.float32)

    wff_pad = np.zeros((FPAD, 1), np.float32)
    wff_pad[:F, 0] = Wf[H2:, 0]

    common = {
        "W1": np.ascontiguousarray(W1.astype(np.float16)),
        "W2": np.ascontiguousarray(W2.astype(np.float16)),
        "Tm": np.ascontiguousarray(T.astype(np.float16)),
        "Wfh": np.ascontiguousarray(Wf[:H2].reshape(H2, 1).astype(np.float16)),
    }
    # packed constants (per-core: iotas and wcol differ)
    cst0 = np.zeros((P, CW), np.float32)
    # iotac: row-major sorted junk values — globally sorted sequences are
    # invariant under the merge network, so each junk partition keeps 512
    # distinct decodable indices
    cst0[:, C_IOTC:C_IOTC + NC] = (
        (np.arange(P, dtype=np.float32)[:, None] // 32) * NC
        + np.arange(NC, dtype=np.float32)[None, :]) / 2048.0
    cst0[:, C_B1:C_B1 + 4] = b1.reshape(4, P).T
    cst0[:, C_B2:C_B2 + 2] = b2.reshape(2, P).T
    cst0[:, C_BF] = bf[0]
    in_maps = []
    for d in range(NCORES):
        m = dict(common)
        m["xT"] = np.ascontiguousarray(
            x[d * BS:(d + 1) * BS, :].T.astype(np.float16))
        cstd = cst0.copy()
        cstd[:, 0:BS] = ((d * BS + np.arange(BS, dtype=np.float32))
                         / 2048.0)[None, :]
        wcol = np.zeros(P, np.float32)
        for q in range(4):
            wcol[32 * q:32 * q + FL] = wff_pad[d * FL:(d + 1) * FL, 0]
        cstd[:, C_WC] = wcol
        m["cst"] = np.ascontiguousarray(cstd)
        in_maps.append(m)
    return in_maps


_NC_CACHE = None


def _get_program():
    global _NC_CACHE
    if _NC_CACHE is None:
        _NC_CACHE = build_program()
    return _NC_CACHE


def kernel(x, W1, b1, W2, b2, T, Wf, bf):
    nc = _get_program()
    in_maps = _build_in_maps(dict(
        x=x, W1=W1, b1=b1, W2=W2, b2=b2, T=T, Wf=Wf, bf=bf))
    res = run_bass_kernel_spmd(nc, in_maps, core_ids=list(range(NCORES)))
    total = np.zeros(B, np.float64)
    for d in range(NCORES):
        total += res.results[d]["out"].ravel().astype(np.float64)
        total[d * BS:(d + 1) * BS] += res.results[d]["outh"].ravel()
    return total.reshape(B, 1).astype(np.float32)

